# revision 54
# baseline (speedup 1.0000x reference)
"""Trainium2 Bass kernel for masked BasicBlock (conv3x3+BN+ReLU, gated, x2, residual).

Data-parallel over batch: 8 images -> 8 NeuronCores. Per core, NCHW [64,256,256]
in 8 row-strips of 32 output rows:
  - The gate_max mask of the reference is mathematically redundant
    (g*conv2(h*maxpool3(g)) == g*conv2(h)); only the final gate g is applied.
    Out-of-image h rows (h[-1], h[256..]) are zeroed explicitly so conv2 sees
    zero padding.
  - conv3x3 = 5 accumulated matmuls per 2-row group over C_in=64:
      3x K=128 "column pairs" (dy, dx=-1)+(dy, dx=+1) via a 2-col-shifted
        duplicate of the input in partitions 64..127 (tile U / Hu),
      1x K=128 "row pair" (dy=-1, dx=0)+(dy=+1, dx=0) via a 2-row-shifted
        duplicate (tile T1 / H1),
      1x K=64 center tap.
    Chunk pairs (lo|hi = 4 consecutive rows) use the two PE column groups via
    tile_position (0,0)/(0,64).
  - The 2-row shift also makes T1[0:128] directly usable as the residual pair.
  - Gate mask broadcast to all partitions with a stride-0 source AP DMA
    (no PE involvement).
  - BN(+ReLU) on ScalarE from PSUM; elementwise gating and residual on VectorE.
  - Software pipelined: the x staging chain for strip s+1 is prefetched on the
    Pool/SWDGE queues while conv1 of strip s runs, and conv1 of strip s is
    issued before conv2 of strip s-1 so the PE never waits on the h-distribute
    DMAs. First/last strips use a T1/H1-only 6-pass conv form plus per-pair
    distribute/stores to shorten pipeline fill and drain.
"""
import sys
import os

sys.path.insert(0, '/opt/trn_rl_repo')

import numpy as np
import ml_dtypes

BF16 = ml_dtypes.bfloat16

B, C, H, W = 8, 64, 256, 256
WP = W + 2           # padded row width
R = 32               # output rows per strip
NS = H // R          # strips
NP1 = (R + 4) // 4   # conv1 pairs per strip (h rows r0-1 .. r0+34)
NP2 = R // 4         # conv2 pairs per strip
XR = R + 6           # x rows per strip: [r0-2, r0+36)
HR = R + 4           # h rows per strip: [r0-1, r0+35)
PAD = 4              # zero rows padded above/below the flat mask
GTR = R + 1          # gate flat rows loaded per strip

_CACHE = {}


def _build(iters=1):
    import concourse.bacc as bacc_mod
    import concourse.tile as tile
    import concourse.mybir as mybir
    from concourse.bass import AP

    dt = mybir.dt
    nc = bacc_mod.Bacc()

    x_d = nc.dram_tensor("x", [C, H, W], dt.float32, kind="ExternalInput")
    gt_d = nc.dram_tensor("gt", [(H + 2 * PAD) * W], dt.bfloat16, kind="ExternalInput")
    wc1_d = nc.dram_tensor("wc1", [128, 3, 64], dt.bfloat16, kind="ExternalInput")
    wr1_d = nc.dram_tensor("wr1", [128, 64], dt.bfloat16, kind="ExternalInput")
    wce1_d = nc.dram_tensor("wce1", [64, 64], dt.bfloat16, kind="ExternalInput")
    wc2_d = nc.dram_tensor("wc2", [128, 3, 64], dt.bfloat16, kind="ExternalInput")
    wr2_d = nc.dram_tensor("wr2", [128, 64], dt.bfloat16, kind="ExternalInput")
    wce2_d = nc.dram_tensor("wce2", [64, 64], dt.bfloat16, kind="ExternalInput")
    wp1_d = nc.dram_tensor("wp1", [128, 3, 64], dt.bfloat16, kind="ExternalInput")
    ws1_d = nc.dram_tensor("ws1", [64, 3, 64], dt.bfloat16, kind="ExternalInput")
    wp2_d = nc.dram_tensor("wp2", [128, 3, 64], dt.bfloat16, kind="ExternalInput")
    ws2_d = nc.dram_tensor("ws2", [64, 3, 64], dt.bfloat16, kind="ExternalInput")
    sb1_d = nc.dram_tensor("sb1", [128, 2], dt.float32, kind="ExternalInput")
    sb2_d = nc.dram_tensor("sb2", [128, 2], dt.float32, kind="ExternalInput")
    o_d = nc.dram_tensor("o", [C, H, W], dt.bfloat16, kind="ExternalOutput")

    with tile.TileContext(nc) as tc:
        with (
            tc.tile_pool(name="const", bufs=1) as cpool,
            tc.tile_pool(name="xs", bufs=3) as xpool,
            tc.tile_pool(name="us", bufs=2) as upool,
            tc.tile_pool(name="hs", bufs=2) as hpool,
            tc.tile_pool(name="hu", bufs=2) as hupool,
            tc.tile_pool(name="stage", bufs=1) as spool,
            tc.tile_pool(name="ov", bufs=1) as ovpool,
            tc.tile_pool(name="flat", bufs=1) as fpool,
            tc.tile_pool(name="pair", bufs=3) as ppool,
            tc.tile_pool(name="ps1", bufs=4, space="PSUM") as ps1,
            tc.tile_pool(name="ps2", bufs=4, space="PSUM") as ps2,
        ):
            wc1 = cpool.tile([128, 3, 64], dt.bfloat16)
            wr1 = cpool.tile([128, 64], dt.bfloat16)
            wce1 = cpool.tile([64, 64], dt.bfloat16)
            wc2 = cpool.tile([128, 3, 64], dt.bfloat16)
            wr2 = cpool.tile([128, 64], dt.bfloat16)
            wce2 = cpool.tile([64, 64], dt.bfloat16)
            wp1 = cpool.tile([128, 3, 64], dt.bfloat16)
            ws1 = cpool.tile([64, 3, 64], dt.bfloat16)
            wp2 = cpool.tile([128, 3, 64], dt.bfloat16)
            ws2 = cpool.tile([64, 3, 64], dt.bfloat16)
            sb1 = cpool.tile([128, 2], dt.float32)
            sb2 = cpool.tile([128, 2], dt.float32)
            nc.sync.dma_start(wp1[:], wp1_d[:])
            nc.sync.dma_start(ws1[:], ws1_d[:])
            nc.sync.dma_start(wp2[:], wp2_d[:])
            nc.sync.dma_start(ws2[:], ws2_d[:])
            nc.sync.dma_start(wc1[:], wc1_d[:])
            nc.sync.dma_start(wr1[:], wr1_d[:])
            nc.sync.dma_start(wce1[:], wce1_d[:])
            nc.sync.dma_start(wc2[:], wc2_d[:])
            nc.sync.dma_start(wr2[:], wr2_d[:])
            nc.sync.dma_start(wce2[:], wce2_d[:])
            nc.sync.dma_start(sb1[:], sb1_d[:])
            nc.sync.dma_start(sb2[:], sb2_d[:])

            NT = iters * NS

            # x strip staging, all on the Pool queue so the FIFO order matches
            # the dependency chain: load -> row-shift -> col-shift copies.
            # T1 [128, XR, WP] bf16: lower=x padded, upper=x shifted +2 rows.
            # U  [128, XR, 256] bf16: lower=x, upper=x shifted +2 cols.
            def emit_xchain(s):
                r0 = s * R
                T1 = xpool.tile([128, XR, WP], dt.bfloat16, tag="T1")
                first = r0 - 2
                v0 = max(0, -first)
                v1 = min(XR, H - first)
                nc.vector.memset(T1[0:64, :, 0:1], 0)
                nc.vector.memset(T1[0:64, :, 257:258], 0)
                if v0 > 0:
                    nc.vector.memset(T1[0:64, 0:v0, :], 0)
                if v1 < XR:
                    nc.vector.memset(T1[0:64, v1:XR, :], 0)
                if s == 0:
                    # split the first load and shift so the first pairs'
                    # matmuls can start before the whole strip has landed
                    prev_r = v0
                    for mid in (7, 16, 26, XR):
                        nc.gpsimd.dma_start(T1[0:64, prev_r:mid, 1:257], x_d[:, first + prev_r:first + mid, :])
                        nc.gpsimd.dma_start(T1[64:128, max(0, prev_r - 2):mid - 2, :],
                                            T1[0:64, max(2, prev_r):mid, :])
                        prev_r = mid
                    return T1, None  # strip 0 conv1 is 6-pass (T1-only)
                nc.gpsimd.dma_start(T1[0:64, v0:v1, 1:257], x_d[:, first + v0:first + v1, :])
                nc.gpsimd.dma_start(T1[64:128, 0:XR - 2, :], T1[0:64, 2:XR, :])
                if s == 1:
                    return T1, None  # strip 1 conv1 is 6-pass too (lighter fill)
                U = upool.tile([128, XR, 256], dt.bfloat16, tag="U")
                nc.gpsimd.dma_start(U[0:64, :, :], T1[0:64, :, 0:256])
                nc.gpsimd.dma_start(U[64:128, :, :], T1[0:64, :, 2:258])
                return T1, U

            # gate tile [128, NP2, 512]: partition-broadcast via stride-0 src AP;
            # lower partitions get each pair's first 2 rows, upper the next 2
            def emit_gate(s):
                gts = fpool.tile([128, NP2, 512], dt.bfloat16, tag="gts")
                gt0 = (s * R + PAD) * W
                nc.sync.dma_start(gts[0:64, :, :],
                                  AP(gt_d, gt0, [[0, 64], [1024, NP2], [1, 512]]))
                nc.sync.dma_start(gts[64:128, :, :],
                                  AP(gt_d, gt0 + 512, [[0, 64], [1024, NP2], [1, 512]]))
                return gts

            gts_cur = emit_gate(0)

            pending = emit_xchain(0)
            prev = None  # (T1, H1, Hu, s, r0) of strip awaiting conv2
            for it in range(NT + 1):
                if it < NT:
                    s = it % NS
                    r0 = s * R
                    T1, U = pending
                    if it + 1 < NT:
                        pending = emit_xchain((it + 1) % NS)

                    # ---- conv1: 9 pairs of 4 h-rows, 5 passes each half
                    HP = spool.tile([128, NP1, 512], dt.bfloat16, tag="HP")
                    H1 = hpool.tile([128, HR, WP], dt.bfloat16, tag="H1")
                    nc.vector.memset(H1[:, :, 0:1], 0)
                    nc.vector.memset(H1[:, :, 257:258], 0)
                    pp_dist = s in (0, NS - 1)
                    if s > 0:
                        # first 4 h rows (r0-1..r0+3) were already computed by
                        # the previous strip; copy instead of recomputing
                        H1prev = prev[1]
                        nc.sync.dma_start(H1[0:64, 0:4, :], H1prev[0:64, 32:36, :])
                        nc.sync.dma_start(H1[64:128, 0:2, :], H1prev[64:128, 32:34, :])
                    for pq in range(1 if s > 0 else 0, NP1):
                        acc = ps1.tile([128, 512], dt.float32, tag="ps1")
                        b = 4 * pq
                        if s == 0:
                            # T1-only 6-pass form, lower-partition rows first:
                            # no wait on the U col-shift chain or the row-shift
                            # right at kernel start
                            for dx in range(3):
                                nc.tensor.matmul(acc[0:64, :], ws1[:, dx, :], T1[0:64, b + 1:b + 3, dx:dx + 256],
                                                 start=(dx == 0), stop=False, tile_position=(0, 0), skip_group_check=True)
                                nc.tensor.matmul(acc[64:128, :], ws1[:, dx, :], T1[0:64, b + 3:b + 5, dx:dx + 256],
                                                 start=(dx == 0), stop=False, tile_position=(0, 64), skip_group_check=True)
                            for dx in range(3):
                                nc.tensor.matmul(acc[0:64, :], wp1[:, dx, :], T1[:, b:b + 2, dx:dx + 256],
                                                 start=False, stop=(dx == 2), tile_position=(0, 0), skip_group_check=True)
                                nc.tensor.matmul(acc[64:128, :], wp1[:, dx, :], T1[:, b + 2:b + 4, dx:dx + 256],
                                                 start=False, stop=(dx == 2), tile_position=(0, 64), skip_group_check=True)
                            nc.scalar.activation(HP[:, pq, :], acc[:], mybir.ActivationFunctionType.Relu,
                                                 bias=sb1[:, 1:2], scale=sb1[:, 0:1])
                            if pq == 0:
                                nc.vector.memset(HP[0:64, 0, 0:256], 0)   # h row -1
                            hp2 = HP[:, pq, :].rearrange("c (rr w) -> c rr w", rr=2)
                            nc.sync.dma_start(H1[0:64, 4 * pq:4 * pq + 2, 1:257], hp2[0:64])
                            nc.sync.dma_start(H1[0:64, 4 * pq + 2:4 * pq + 4, 1:257], hp2[64:128])
                            if pq > 0:
                                nc.sync.dma_start(H1[64:128, 4 * pq - 2:4 * pq, 1:257], hp2[0:64])
                            nc.sync.dma_start(H1[64:128, 4 * pq:4 * pq + 2, 1:257], hp2[64:128])
                            continue
                        if s == 1:
                            for dx in range(3):
                                nc.tensor.matmul(acc[0:64, :], ws1[:, dx, :], T1[0:64, b + 1:b + 3, dx:dx + 256],
                                                 start=(dx == 0), stop=False, tile_position=(0, 0), skip_group_check=True)
                                nc.tensor.matmul(acc[64:128, :], ws1[:, dx, :], T1[0:64, b + 3:b + 5, dx:dx + 256],
                                                 start=(dx == 0), stop=False, tile_position=(0, 64), skip_group_check=True)
                            for dx in range(3):
                                nc.tensor.matmul(acc[0:64, :], wp1[:, dx, :], T1[:, b:b + 2, dx:dx + 256],
                                                 start=False, stop=(dx == 2), tile_position=(0, 0), skip_group_check=True)
                                nc.tensor.matmul(acc[64:128, :], wp1[:, dx, :], T1[:, b + 2:b + 4, dx:dx + 256],
                                                 start=False, stop=(dx == 2), tile_position=(0, 64), skip_group_check=True)
                        else:
                            nc.tensor.matmul(acc[0:64, :], wce1[:], T1[0:64, b + 1:b + 3, 1:257],
                                             start=True, stop=False, tile_position=(0, 0), skip_group_check=True)
                            nc.tensor.matmul(acc[64:128, :], wce1[:], T1[0:64, b + 3:b + 5, 1:257],
                                             start=True, stop=False, tile_position=(0, 64), skip_group_check=True)
                            nc.tensor.matmul(acc[0:64, :], wr1[:], T1[:, b:b + 2, 1:257],
                                             start=False, stop=False, tile_position=(0, 0), skip_group_check=True)
                            nc.tensor.matmul(acc[64:128, :], wr1[:], T1[:, b + 2:b + 4, 1:257],
                                             start=False, stop=False, tile_position=(0, 64), skip_group_check=True)
                            for k in range(3):
                                nc.tensor.matmul(acc[0:64, :], wc1[:, k, :], U[:, b + k:b + k + 2, :],
                                                 start=False, stop=(k == 2), tile_position=(0, 0), skip_group_check=True)
                                nc.tensor.matmul(acc[64:128, :], wc1[:, k, :], U[:, b + 2 + k:b + 4 + k, :],
                                                 start=False, stop=(k == 2), tile_position=(0, 64), skip_group_check=True)
                        nc.scalar.activation(HP[:, pq, :], acc[:], mybir.ActivationFunctionType.Relu,
                                             bias=sb1[:, 1:2], scale=sb1[:, 0:1])
                        # zero h rows outside the image so conv2 sees zero padding
                        if s == NS - 1 and pq == NP1 - 1:
                            nc.vector.memset(HP[0:64, NP1 - 1, 256:512], 0)  # h row 256
                            nc.vector.memset(HP[64:128, NP1 - 1, :], 0)      # h rows 257,258
                        if pp_dist:
                            hp2 = HP[:, pq, :].rearrange("c (rr w) -> c rr w", rr=2)
                            nc.sync.dma_start(H1[0:64, 4 * pq:4 * pq + 2, 1:257], hp2[0:64])
                            nc.sync.dma_start(H1[0:64, 4 * pq + 2:4 * pq + 4, 1:257], hp2[64:128])
                            if pq > 0:
                                nc.sync.dma_start(H1[64:128, 4 * pq - 2:4 * pq, 1:257], hp2[0:64])
                            nc.sync.dma_start(H1[64:128, 4 * pq:4 * pq + 2, 1:257], hp2[64:128])

                    if not pp_dist:
                        # distribute HP -> H1 for pairs 1..NP1-1 (lower = h,
                        # upper = h shifted +2 rows); DMA APs max 3 dims: one
                        # DMA per row-in-pair
                        np1 = NP1 - 1
                        h1v = H1[0:64, 4:4 + 4 * np1, 1:257].rearrange("c (p rr) w -> c p rr w", p=np1)
                        h1u = H1[64:128, 2:2 + 4 * np1, 1:257].rearrange("c (p rr) w -> c p rr w", p=np1)
                        h1u2 = H1[64:128, 4:4 + 4 * np1, 1:257].rearrange("c (p rr) w -> c p rr w", p=np1)
                        hpv = HP[:].rearrange("c p (rr w) -> c p rr w", rr=2)
                        for rr in range(2):
                            nc.sync.dma_start(h1v[:, :, rr, :], hpv[0:64, 1:NP1, rr, :])
                            nc.sync.dma_start(h1v[:, :, 2 + rr, :], hpv[64:128, 1:NP1, rr, :])
                            nc.sync.dma_start(h1u[:, :, rr, :], hpv[0:64, 1:NP1, rr, :])
                            nc.sync.dma_start(h1u2[:, :, rr, :], hpv[64:128, 1:NP1, rr, :])
                    if s != NS - 1:
                        # Hu: lower = h, upper = h shifted +2 cols (5-pass conv2)
                        Hu = hupool.tile([128, HR, 256], dt.bfloat16, tag="Hu")
                        nc.sync.dma_start(Hu[0:64, :, :], H1[0:64, :, 0:256])
                        nc.sync.dma_start(Hu[64:128, :, :], H1[0:64, :, 2:258])
                    else:
                        Hu = None

                    cur = (T1, H1, Hu, s, r0)
                else:
                    cur = None

                if prev is not None:
                    T1p, H1p, Hup, sp, r0p = prev
                    # ---- conv2: 8 pairs of 4 output rows, 5 passes each half
                    OV = ovpool.tile([128, NP2, 512], dt.bfloat16, tag="OV")
                    for q in range(NP2):
                        b = 4 * q
                        acc2 = ps2.tile([128, 512], dt.float32, tag="ps2")
                        if sp == NS - 1:
                            # H1-only 6-pass form: no wait on the Hu col-shift
                            # chain in the pipeline drain
                            for dx in range(3):
                                nc.tensor.matmul(acc2[0:64, :], wp2[:, dx, :], H1p[:, b:b + 2, dx:dx + 256],
                                                 start=(dx == 0), stop=False, tile_position=(0, 0), skip_group_check=True)
                                nc.tensor.matmul(acc2[64:128, :], wp2[:, dx, :], H1p[:, b + 2:b + 4, dx:dx + 256],
                                                 start=(dx == 0), stop=False, tile_position=(0, 64), skip_group_check=True)
                            for dx in range(3):
                                nc.tensor.matmul(acc2[0:64, :], ws2[:, dx, :], H1p[0:64, b + 1:b + 3, dx:dx + 256],
                                                 start=False, stop=(dx == 2), tile_position=(0, 0), skip_group_check=True)
                                nc.tensor.matmul(acc2[64:128, :], ws2[:, dx, :], H1p[0:64, b + 3:b + 5, dx:dx + 256],
                                                 start=False, stop=(dx == 2), tile_position=(0, 64), skip_group_check=True)
                        else:
                            nc.tensor.matmul(acc2[0:64, :], wce2[:], H1p[0:64, b + 1:b + 3, 1:257],
                                             start=True, stop=False, tile_position=(0, 0), skip_group_check=True)
                            nc.tensor.matmul(acc2[64:128, :], wce2[:], H1p[0:64, b + 3:b + 5, 1:257],
                                             start=True, stop=False, tile_position=(0, 64), skip_group_check=True)
                            nc.tensor.matmul(acc2[0:64, :], wr2[:], H1p[:, b:b + 2, 1:257],
                                             start=False, stop=False, tile_position=(0, 0), skip_group_check=True)
                            nc.tensor.matmul(acc2[64:128, :], wr2[:], H1p[:, b + 2:b + 4, 1:257],
                                             start=False, stop=False, tile_position=(0, 64), skip_group_check=True)
                            for k in range(3):
                                nc.tensor.matmul(acc2[0:64, :], wc2[:, k, :], Hup[:, b + k:b + k + 2, :],
                                                 start=False, stop=(k == 2), tile_position=(0, 0), skip_group_check=True)
                                nc.tensor.matmul(acc2[64:128, :], wc2[:, k, :], Hup[:, b + 2 + k:b + 4 + k, :],
                                                 start=False, stop=(k == 2), tile_position=(0, 64), skip_group_check=True)

                        u2 = ppool.tile([128, 512], dt.float32, tag="u2")
                        nc.scalar.activation(u2[:], acc2[:], mybir.ActivationFunctionType.Identity,
                                             bias=sb2[:, 1:2], scale=sb2[:, 0:1])
                        nc.vector.tensor_tensor(u2[:], u2[:], gts_cur[:, q, :], mybir.AluOpType.mult)
                        lz = b + 2
                        nc.vector.tensor_tensor(u2[:].rearrange("p (r w) -> p r w", r=2),
                                                u2[:].rearrange("p (r w) -> p r w", r=2),
                                                T1p[:, lz:lz + 2, 1:257], mybir.AluOpType.add)
                        nc.scalar.activation(OV[:, q, :], u2[:], mybir.ActivationFunctionType.Relu)
                        if sp == NS - 1:
                            ovq = OV[:, q, :].rearrange("c (rr w) -> c rr w", rr=2)
                            nc.sync.dma_start(o_d[:, r0p + b:r0p + b + 2, :], ovq[0:64])
                            nc.sync.dma_start(o_d[:, r0p + b + 2:r0p + b + 4, :], ovq[64:128])

                    if sp != NS - 1:
                        ov = o_d[:, r0p:r0p + R, :].rearrange("c (p rr) w -> c p rr w", p=NP2)
                        ovv = OV[:].rearrange("c p (rr w) -> c p rr w", rr=2)
                        for rr in range(2):
                            nc.scalar.dma_start(ov[:, :, rr, :], ovv[0:64, :, rr, :])
                            nc.scalar.dma_start(ov[:, :, 2 + rr, :], ovv[64:128, :, rr, :])

                if 0 < it < NT:
                    # gate for iteration it+1's conv2 (bufs=1: WAR on this
                    # iteration's conv2 mask reads orders this load after them)
                    gts_cur = emit_gate(it % NS)

                prev = cur
    nc.finalize()
    return nc


def _host_prep(gate, w1, scale1, bias1, w2, scale2, bias2):
    # lhsT[ci, co] = w[co, ci, dy, dx]; K-pack (dy, dx=-1)+(dy, dx=+1) into wc,
    # (dy=-1, 0)+(dy=+1, 0) into wr; center tap alone in wce
    def pack(w):
        wt = np.transpose(w, (1, 0, 2, 3))  # [ci, co, dy, dx]
        wc = np.empty((128, 3, 64), np.float32)
        for k in range(3):
            wc[0:64, k] = wt[:, :, k, 0]
            wc[64:128, k] = wt[:, :, k, 2]
        wr = np.concatenate([wt[:, :, 0, 1], wt[:, :, 2, 1]], axis=0)
        wce = wt[:, :, 1, 1]
        return wc.astype(BF16), wr.astype(BF16), wce.astype(BF16)

    # baseline 6-pass layout: K-pack dy=-1 (lower) with dy=+1 (upper) per dx
    def pack6(w):
        wt = np.transpose(w, (1, 0, 2, 3))  # [ci, co, dy, dx]
        wp = np.empty((128, 3, 64), np.float32)
        ws = np.empty((64, 3, 64), np.float32)
        for dx in range(3):
            wp[0:64, dx] = wt[:, :, 0, dx]
            wp[64:128, dx] = wt[:, :, 2, dx]
            ws[:, dx] = wt[:, :, 1, dx]
        return wp.astype(BF16), ws.astype(BF16)

    wc1, wr1, wce1 = pack(w1)
    wc2, wr2, wce2 = pack(w2)
    wp1, ws1 = pack6(w1)
    wp2, ws2 = pack6(w2)
    sb1 = np.stack([np.tile(scale1, 2), np.tile(bias1, 2)], axis=1).astype(np.float32)
    sb2 = np.stack([np.tile(scale2, 2), np.tile(bias2, 2)], axis=1).astype(np.float32)

    def flat_padded(m):
        mp = np.zeros((H + 2 * PAD, W), np.float32)
        mp[PAD:PAD + H] = m
        return mp.reshape(-1).astype(BF16)

    gt_list = [flat_padded(gate[b, 0]) for b in range(B)]
    return (wc1, wr1, wce1, wc2, wr2, wce2, wp1, ws1, wp2, ws2,
            sb1, sb2, gt_list)


def kernel(x, gate, w1, scale1, bias1, w2, scale2, bias2):
    from concourse.bass_utils import run_bass_kernel_spmd

    x = np.asarray(x, np.float32)
    gate = np.asarray(gate, np.float32)
    (wc1, wr1, wce1, wc2, wr2, wce2, wp1, ws1, wp2, ws2,
     sb1, sb2, gt_list) = _host_prep(
        gate, np.asarray(w1, np.float32), np.asarray(scale1, np.float32),
        np.asarray(bias1, np.float32), np.asarray(w2, np.float32),
        np.asarray(scale2, np.float32), np.asarray(bias2, np.float32))

    if 'nc' not in _CACHE:
        _CACHE['nc'] = _build()
    nc = _CACHE['nc']

    in_maps = []
    for b in range(B):
        in_maps.append({
            "x": np.ascontiguousarray(x[b]),
            "gt": gt_list[b],
            "wc1": wc1, "wr1": wr1, "wce1": wce1,
            "wc2": wc2, "wr2": wr2, "wce2": wce2,
            "wp1": wp1, "ws1": ws1, "wp2": wp2, "ws2": ws2,
            "sb1": sb1, "sb2": sb2,
        })
    res = run_bass_kernel_spmd(nc, in_maps, core_ids=list(range(B)))
    _CACHE['last_results'] = res
    out = np.stack([np.asarray(res.results[b]["o"], np.float32) for b in range(B)], axis=0)
    return out


# revision 55
# speedup vs baseline: 1.0020x; 1.0020x over previous
"""Trainium2 Bass kernel for masked BasicBlock (conv3x3+BN+ReLU, gated, x2, residual).

Data-parallel over batch: 8 images -> 8 NeuronCores. Per core, NCHW [64,256,256]
in 8 row-strips of 32 output rows:
  - The gate_max mask of the reference is mathematically redundant
    (g*conv2(h*maxpool3(g)) == g*conv2(h)); only the final gate g is applied.
    Out-of-image h rows (h[-1], h[256..]) are zeroed explicitly so conv2 sees
    zero padding.
  - conv3x3 = 5 accumulated matmuls per 2-row group over C_in=64:
      3x K=128 "column pairs" (dy, dx=-1)+(dy, dx=+1) via a 2-col-shifted
        duplicate of the input in partitions 64..127 (tile U / Hu),
      1x K=128 "row pair" (dy=-1, dx=0)+(dy=+1, dx=0) via a 2-row-shifted
        duplicate (tile T1 / H1),
      1x K=64 center tap.
    Chunk pairs (lo|hi = 4 consecutive rows) use the two PE column groups via
    tile_position (0,0)/(0,64).
  - The 2-row shift also makes T1[0:128] directly usable as the residual pair.
  - Gate mask broadcast to all partitions with a stride-0 source AP DMA
    (no PE involvement).
  - BN(+ReLU) on ScalarE from PSUM; elementwise gating and residual on VectorE.
  - Software pipelined: the x staging chain for strip s+1 is prefetched on the
    Pool/SWDGE queues while conv1 of strip s runs, and conv1 of strip s is
    issued before conv2 of strip s-1 so the PE never waits on the h-distribute
    DMAs. First/last strips use a T1/H1-only 6-pass conv form plus per-pair
    distribute/stores to shorten pipeline fill and drain.
"""
import sys
import os

sys.path.insert(0, '/opt/trn_rl_repo')

import numpy as np
import ml_dtypes

BF16 = ml_dtypes.bfloat16

B, C, H, W = 8, 64, 256, 256
WP = W + 2           # padded row width
R = 32               # output rows per strip
NS = H // R          # strips
NP1 = (R + 4) // 4   # conv1 pairs per strip (h rows r0-1 .. r0+34)
NP2 = R // 4         # conv2 pairs per strip
XR = R + 6           # x rows per strip: [r0-2, r0+36)
HR = R + 4           # h rows per strip: [r0-1, r0+35)
PAD = 4              # zero rows padded above/below the flat mask
GTR = R + 1          # gate flat rows loaded per strip

_CACHE = {}


def _build(iters=1):
    import concourse.bacc as bacc_mod
    import concourse.tile as tile
    import concourse.mybir as mybir
    from concourse.bass import AP

    dt = mybir.dt
    nc = bacc_mod.Bacc()

    x_d = nc.dram_tensor("x", [C, H, W], dt.float32, kind="ExternalInput")
    gt_d = nc.dram_tensor("gt", [(H + 2 * PAD) * W], dt.bfloat16, kind="ExternalInput")
    wc1_d = nc.dram_tensor("wc1", [128, 3, 64], dt.bfloat16, kind="ExternalInput")
    wr1_d = nc.dram_tensor("wr1", [128, 64], dt.bfloat16, kind="ExternalInput")
    wce1_d = nc.dram_tensor("wce1", [64, 64], dt.bfloat16, kind="ExternalInput")
    wc2_d = nc.dram_tensor("wc2", [128, 3, 64], dt.bfloat16, kind="ExternalInput")
    wr2_d = nc.dram_tensor("wr2", [128, 64], dt.bfloat16, kind="ExternalInput")
    wce2_d = nc.dram_tensor("wce2", [64, 64], dt.bfloat16, kind="ExternalInput")
    wp1_d = nc.dram_tensor("wp1", [128, 3, 64], dt.bfloat16, kind="ExternalInput")
    ws1_d = nc.dram_tensor("ws1", [64, 3, 64], dt.bfloat16, kind="ExternalInput")
    wp2_d = nc.dram_tensor("wp2", [128, 3, 64], dt.bfloat16, kind="ExternalInput")
    ws2_d = nc.dram_tensor("ws2", [64, 3, 64], dt.bfloat16, kind="ExternalInput")
    sb1_d = nc.dram_tensor("sb1", [128, 2], dt.float32, kind="ExternalInput")
    sb2_d = nc.dram_tensor("sb2", [128, 2], dt.float32, kind="ExternalInput")
    o_d = nc.dram_tensor("o", [C, H, W], dt.bfloat16, kind="ExternalOutput")

    with tile.TileContext(nc) as tc:
        with (
            tc.tile_pool(name="const", bufs=1) as cpool,
            tc.tile_pool(name="xs", bufs=3) as xpool,
            tc.tile_pool(name="us", bufs=2) as upool,
            tc.tile_pool(name="hs", bufs=2) as hpool,
            tc.tile_pool(name="hu", bufs=2) as hupool,
            tc.tile_pool(name="stage", bufs=1) as spool,
            tc.tile_pool(name="ov", bufs=1) as ovpool,
            tc.tile_pool(name="flat", bufs=1) as fpool,
            tc.tile_pool(name="pair", bufs=3) as ppool,
            tc.tile_pool(name="ps1", bufs=4, space="PSUM") as ps1,
            tc.tile_pool(name="ps2", bufs=4, space="PSUM") as ps2,
        ):
            wc1 = cpool.tile([128, 3, 64], dt.bfloat16)
            wr1 = cpool.tile([128, 64], dt.bfloat16)
            wce1 = cpool.tile([64, 64], dt.bfloat16)
            wc2 = cpool.tile([128, 3, 64], dt.bfloat16)
            wr2 = cpool.tile([128, 64], dt.bfloat16)
            wce2 = cpool.tile([64, 64], dt.bfloat16)
            wp1 = cpool.tile([128, 3, 64], dt.bfloat16)
            ws1 = cpool.tile([64, 3, 64], dt.bfloat16)
            wp2 = cpool.tile([128, 3, 64], dt.bfloat16)
            ws2 = cpool.tile([64, 3, 64], dt.bfloat16)
            sb1 = cpool.tile([128, 2], dt.float32)
            sb2 = cpool.tile([128, 2], dt.float32)
            nc.sync.dma_start(wp1[:], wp1_d[:])
            nc.sync.dma_start(ws1[:], ws1_d[:])
            nc.sync.dma_start(wp2[:], wp2_d[:])
            nc.sync.dma_start(ws2[:], ws2_d[:])
            nc.sync.dma_start(wc1[:], wc1_d[:])
            nc.sync.dma_start(wr1[:], wr1_d[:])
            nc.sync.dma_start(wce1[:], wce1_d[:])
            nc.sync.dma_start(wc2[:], wc2_d[:])
            nc.sync.dma_start(wr2[:], wr2_d[:])
            nc.sync.dma_start(wce2[:], wce2_d[:])
            nc.sync.dma_start(sb1[:], sb1_d[:])
            nc.sync.dma_start(sb2[:], sb2_d[:])

            NT = iters * NS

            # x strip staging, all on the Pool queue so the FIFO order matches
            # the dependency chain: load -> row-shift -> col-shift copies.
            # T1 [128, XR, WP] bf16: lower=x padded, upper=x shifted +2 rows.
            # U  [128, XR, 256] bf16: lower=x, upper=x shifted +2 cols.
            def emit_xchain(s):
                r0 = s * R
                T1 = xpool.tile([128, XR, WP], dt.bfloat16, tag="T1")
                first = r0 - 2
                v0 = max(0, -first)
                v1 = min(XR, H - first)
                nc.vector.memset(T1[0:64, :, 0:1], 0)
                nc.vector.memset(T1[0:64, :, 257:258], 0)
                if v0 > 0:
                    nc.vector.memset(T1[0:64, 0:v0, :], 0)
                if v1 < XR:
                    nc.vector.memset(T1[0:64, v1:XR, :], 0)
                if s == 0:
                    # split the first load and shift so the first pairs'
                    # matmuls can start before the whole strip has landed
                    prev_r = v0
                    for mid in (7, 16, 26, XR):
                        nc.gpsimd.dma_start(T1[0:64, prev_r:mid, 1:257], x_d[:, first + prev_r:first + mid, :])
                        nc.gpsimd.dma_start(T1[64:128, max(0, prev_r - 2):mid - 2, :],
                                            T1[0:64, max(2, prev_r):mid, :])
                        prev_r = mid
                    return T1, None  # strip 0 conv1 is 6-pass (T1-only)
                nc.gpsimd.dma_start(T1[0:64, v0:v1, 1:257], x_d[:, first + v0:first + v1, :])
                nc.gpsimd.dma_start(T1[64:128, 0:XR - 2, :], T1[0:64, 2:XR, :])
                if s == 1:
                    return T1, None  # strip 1 conv1 is 6-pass too (lighter fill)
                U = upool.tile([128, XR, 256], dt.bfloat16, tag="U")
                nc.gpsimd.dma_start(U[0:64, :, :], T1[0:64, :, 0:256])
                nc.gpsimd.dma_start(U[64:128, :, :], T1[0:64, :, 2:258])
                return T1, U

            # gate tile [128, NP2, 512]: partition-broadcast via stride-0 src AP;
            # lower partitions get each pair's first 2 rows, upper the next 2
            def emit_gate(s):
                gts = fpool.tile([128, NP2, 512], dt.bfloat16, tag="gts")
                gt0 = (s * R + PAD) * W
                nc.sync.dma_start(gts[0:64, :, :],
                                  AP(gt_d, gt0, [[0, 64], [1024, NP2], [1, 512]]))
                nc.sync.dma_start(gts[64:128, :, :],
                                  AP(gt_d, gt0 + 512, [[0, 64], [1024, NP2], [1, 512]]))
                return gts

            gts_cur = emit_gate(0)

            pending = emit_xchain(0)
            prev = None  # (T1, H1, Hu, s, r0) of strip awaiting conv2
            for it in range(NT + 1):
                if it < NT:
                    s = it % NS
                    r0 = s * R
                    T1, U = pending
                    if it + 1 < NT:
                        pending = emit_xchain((it + 1) % NS)

                    # ---- conv1: 9 pairs of 4 h-rows, 5 passes each half
                    HP = spool.tile([128, NP1, 512], dt.bfloat16, tag="HP")
                    H1 = hpool.tile([128, HR, WP], dt.bfloat16, tag="H1")
                    nc.vector.memset(H1[:, :, 0:1], 0)
                    nc.vector.memset(H1[:, :, 257:258], 0)
                    pp_dist = s in (0, NS - 1)
                    if s > 0:
                        # first 4 h rows (r0-1..r0+3) were already computed by
                        # the previous strip; copy instead of recomputing
                        H1prev = prev[1]
                        nc.sync.dma_start(H1[0:64, 0:4, :], H1prev[0:64, 32:36, :])
                        nc.sync.dma_start(H1[64:128, 0:2, :], H1prev[64:128, 32:34, :])
                    for pq in range(1 if s > 0 else 0, NP1):
                        acc = ps1.tile([128, 512], dt.float32, tag="ps1")
                        b = 4 * pq
                        if s == 0:
                            # T1-only 6-pass form, lower-partition rows first:
                            # no wait on the U col-shift chain or the row-shift
                            # right at kernel start
                            for dx in range(3):
                                nc.tensor.matmul(acc[0:64, :], ws1[:, dx, :], T1[0:64, b + 1:b + 3, dx:dx + 256],
                                                 start=(dx == 0), stop=False, tile_position=(0, 0), skip_group_check=True)
                                nc.tensor.matmul(acc[64:128, :], ws1[:, dx, :], T1[0:64, b + 3:b + 5, dx:dx + 256],
                                                 start=(dx == 0), stop=False, tile_position=(0, 64), skip_group_check=True)
                            for dx in range(3):
                                nc.tensor.matmul(acc[0:64, :], wp1[:, dx, :], T1[:, b:b + 2, dx:dx + 256],
                                                 start=False, stop=(dx == 2), tile_position=(0, 0), skip_group_check=True)
                                nc.tensor.matmul(acc[64:128, :], wp1[:, dx, :], T1[:, b + 2:b + 4, dx:dx + 256],
                                                 start=False, stop=(dx == 2), tile_position=(0, 64), skip_group_check=True)
                            nc.scalar.activation(HP[:, pq, :], acc[:], mybir.ActivationFunctionType.Relu,
                                                 bias=sb1[:, 1:2], scale=sb1[:, 0:1])
                            if pq == 0:
                                nc.vector.memset(HP[0:64, 0, 0:256], 0)   # h row -1
                            hp2 = HP[:, pq, :].rearrange("c (rr w) -> c rr w", rr=2)
                            nc.sync.dma_start(H1[0:64, 4 * pq:4 * pq + 2, 1:257], hp2[0:64])
                            nc.sync.dma_start(H1[0:64, 4 * pq + 2:4 * pq + 4, 1:257], hp2[64:128])
                            if pq > 0:
                                nc.sync.dma_start(H1[64:128, 4 * pq - 2:4 * pq, 1:257], hp2[0:64])
                            nc.sync.dma_start(H1[64:128, 4 * pq:4 * pq + 2, 1:257], hp2[64:128])
                            continue
                        if s == 1:
                            for dx in range(3):
                                nc.tensor.matmul(acc[0:64, :], ws1[:, dx, :], T1[0:64, b + 1:b + 3, dx:dx + 256],
                                                 start=(dx == 0), stop=False, tile_position=(0, 0), skip_group_check=True)
                                nc.tensor.matmul(acc[64:128, :], ws1[:, dx, :], T1[0:64, b + 3:b + 5, dx:dx + 256],
                                                 start=(dx == 0), stop=False, tile_position=(0, 64), skip_group_check=True)
                            for dx in range(3):
                                nc.tensor.matmul(acc[0:64, :], wp1[:, dx, :], T1[:, b:b + 2, dx:dx + 256],
                                                 start=False, stop=(dx == 2), tile_position=(0, 0), skip_group_check=True)
                                nc.tensor.matmul(acc[64:128, :], wp1[:, dx, :], T1[:, b + 2:b + 4, dx:dx + 256],
                                                 start=False, stop=(dx == 2), tile_position=(0, 64), skip_group_check=True)
                        else:
                            nc.tensor.matmul(acc[0:64, :], wce1[:], T1[0:64, b + 1:b + 3, 1:257],
                                             start=True, stop=False, tile_position=(0, 0), skip_group_check=True)
                            nc.tensor.matmul(acc[64:128, :], wce1[:], T1[0:64, b + 3:b + 5, 1:257],
                                             start=True, stop=False, tile_position=(0, 64), skip_group_check=True)
                            nc.tensor.matmul(acc[0:64, :], wr1[:], T1[:, b:b + 2, 1:257],
                                             start=False, stop=False, tile_position=(0, 0), skip_group_check=True)
                            nc.tensor.matmul(acc[64:128, :], wr1[:], T1[:, b + 2:b + 4, 1:257],
                                             start=False, stop=False, tile_position=(0, 64), skip_group_check=True)
                            for k in range(3):
                                nc.tensor.matmul(acc[0:64, :], wc1[:, k, :], U[:, b + k:b + k + 2, :],
                                                 start=False, stop=(k == 2), tile_position=(0, 0), skip_group_check=True)
                                nc.tensor.matmul(acc[64:128, :], wc1[:, k, :], U[:, b + 2 + k:b + 4 + k, :],
                                                 start=False, stop=(k == 2), tile_position=(0, 64), skip_group_check=True)
                        nc.scalar.activation(HP[:, pq, :], acc[:], mybir.ActivationFunctionType.Relu,
                                             bias=sb1[:, 1:2], scale=sb1[:, 0:1])
                        # zero h rows outside the image so conv2 sees zero padding
                        if s == NS - 1 and pq == NP1 - 1:
                            nc.vector.memset(HP[0:64, NP1 - 1, 256:512], 0)  # h row 256
                            nc.vector.memset(HP[64:128, NP1 - 1, :], 0)      # h rows 257,258
                        if pp_dist:
                            hp2 = HP[:, pq, :].rearrange("c (rr w) -> c rr w", rr=2)
                            nc.sync.dma_start(H1[0:64, 4 * pq:4 * pq + 2, 1:257], hp2[0:64])
                            nc.sync.dma_start(H1[0:64, 4 * pq + 2:4 * pq + 4, 1:257], hp2[64:128])
                            if pq > 0:
                                nc.sync.dma_start(H1[64:128, 4 * pq - 2:4 * pq, 1:257], hp2[0:64])
                            nc.sync.dma_start(H1[64:128, 4 * pq:4 * pq + 2, 1:257], hp2[64:128])

                    if not pp_dist:
                        # distribute HP -> H1 for pairs 1..NP1-1 (lower = h,
                        # upper = h shifted +2 rows); DMA APs max 3 dims: one
                        # DMA per row-in-pair
                        np1 = NP1 - 1
                        h1v = H1[0:64, 4:4 + 4 * np1, 1:257].rearrange("c (p rr) w -> c p rr w", p=np1)
                        h1u = H1[64:128, 2:2 + 4 * np1, 1:257].rearrange("c (p rr) w -> c p rr w", p=np1)
                        h1u2 = H1[64:128, 4:4 + 4 * np1, 1:257].rearrange("c (p rr) w -> c p rr w", p=np1)
                        hpv = HP[:].rearrange("c p (rr w) -> c p rr w", rr=2)
                        for rr in range(2):
                            nc.sync.dma_start(h1v[:, :, rr, :], hpv[0:64, 1:NP1, rr, :])
                            nc.sync.dma_start(h1v[:, :, 2 + rr, :], hpv[64:128, 1:NP1, rr, :])
                            nc.sync.dma_start(h1u[:, :, rr, :], hpv[0:64, 1:NP1, rr, :])
                            nc.sync.dma_start(h1u2[:, :, rr, :], hpv[64:128, 1:NP1, rr, :])
                    if s != NS - 1:
                        # Hu: lower = h, upper = h shifted +2 cols (5-pass conv2)
                        Hu = hupool.tile([128, HR, 256], dt.bfloat16, tag="Hu")
                        nc.sync.dma_start(Hu[0:64, :, :], H1[0:64, :, 0:256])
                        nc.sync.dma_start(Hu[64:128, :, :], H1[0:64, :, 2:258])
                    else:
                        Hu = None

                    cur = (T1, H1, Hu, s, r0)
                else:
                    cur = None

                if prev is not None:
                    T1p, H1p, Hup, sp, r0p = prev
                    # ---- conv2: 8 pairs of 4 output rows, 5 passes each half
                    OV = ovpool.tile([128, NP2, 512], dt.bfloat16, tag="OV")
                    for q in range(NP2):
                        b = 4 * q
                        acc2 = ps2.tile([128, 512], dt.float32, tag="ps2")
                        if sp == NS - 1:
                            # H1-only 6-pass form: no wait on the Hu col-shift
                            # chain in the pipeline drain
                            for dx in range(3):
                                nc.tensor.matmul(acc2[0:64, :], wp2[:, dx, :], H1p[:, b:b + 2, dx:dx + 256],
                                                 start=(dx == 0), stop=False, tile_position=(0, 0), skip_group_check=True)
                                nc.tensor.matmul(acc2[64:128, :], wp2[:, dx, :], H1p[:, b + 2:b + 4, dx:dx + 256],
                                                 start=(dx == 0), stop=False, tile_position=(0, 64), skip_group_check=True)
                            for dx in range(3):
                                nc.tensor.matmul(acc2[0:64, :], ws2[:, dx, :], H1p[0:64, b + 1:b + 3, dx:dx + 256],
                                                 start=False, stop=(dx == 2), tile_position=(0, 0), skip_group_check=True)
                                nc.tensor.matmul(acc2[64:128, :], ws2[:, dx, :], H1p[0:64, b + 3:b + 5, dx:dx + 256],
                                                 start=False, stop=(dx == 2), tile_position=(0, 64), skip_group_check=True)
                        else:
                            nc.tensor.matmul(acc2[0:64, :], wce2[:], H1p[0:64, b + 1:b + 3, 1:257],
                                             start=True, stop=False, tile_position=(0, 0), skip_group_check=True)
                            nc.tensor.matmul(acc2[64:128, :], wce2[:], H1p[0:64, b + 3:b + 5, 1:257],
                                             start=True, stop=False, tile_position=(0, 64), skip_group_check=True)
                            nc.tensor.matmul(acc2[0:64, :], wr2[:], H1p[:, b:b + 2, 1:257],
                                             start=False, stop=False, tile_position=(0, 0), skip_group_check=True)
                            nc.tensor.matmul(acc2[64:128, :], wr2[:], H1p[:, b + 2:b + 4, 1:257],
                                             start=False, stop=False, tile_position=(0, 64), skip_group_check=True)
                            for k in range(3):
                                nc.tensor.matmul(acc2[0:64, :], wc2[:, k, :], Hup[:, b + k:b + k + 2, :],
                                                 start=False, stop=(k == 2), tile_position=(0, 0), skip_group_check=True)
                                nc.tensor.matmul(acc2[64:128, :], wc2[:, k, :], Hup[:, b + 2 + k:b + 4 + k, :],
                                                 start=False, stop=(k == 2), tile_position=(0, 64), skip_group_check=True)

                        u2 = ppool.tile([128, 512], dt.bfloat16, tag="u2")
                        nc.scalar.activation(u2[:], acc2[:], mybir.ActivationFunctionType.Identity,
                                             bias=sb2[:, 1:2], scale=sb2[:, 0:1])
                        nc.vector.tensor_tensor(u2[:], u2[:], gts_cur[:, q, :], mybir.AluOpType.mult)
                        lz = b + 2
                        nc.vector.tensor_tensor(u2[:].rearrange("p (r w) -> p r w", r=2),
                                                u2[:].rearrange("p (r w) -> p r w", r=2),
                                                T1p[:, lz:lz + 2, 1:257], mybir.AluOpType.add)
                        nc.scalar.activation(OV[:, q, :], u2[:], mybir.ActivationFunctionType.Relu)
                        if sp == NS - 1:
                            ovq = OV[:, q, :].rearrange("c (rr w) -> c rr w", rr=2)
                            nc.sync.dma_start(o_d[:, r0p + b:r0p + b + 2, :], ovq[0:64])
                            nc.sync.dma_start(o_d[:, r0p + b + 2:r0p + b + 4, :], ovq[64:128])

                    if sp != NS - 1:
                        ov = o_d[:, r0p:r0p + R, :].rearrange("c (p rr) w -> c p rr w", p=NP2)
                        ovv = OV[:].rearrange("c p (rr w) -> c p rr w", rr=2)
                        for rr in range(2):
                            nc.scalar.dma_start(ov[:, :, rr, :], ovv[0:64, :, rr, :])
                            nc.scalar.dma_start(ov[:, :, 2 + rr, :], ovv[64:128, :, rr, :])

                if 0 < it < NT:
                    # gate for iteration it+1's conv2 (bufs=1: WAR on this
                    # iteration's conv2 mask reads orders this load after them)
                    gts_cur = emit_gate(it % NS)

                prev = cur
    nc.finalize()
    return nc


def _host_prep(gate, w1, scale1, bias1, w2, scale2, bias2):
    # lhsT[ci, co] = w[co, ci, dy, dx]; K-pack (dy, dx=-1)+(dy, dx=+1) into wc,
    # (dy=-1, 0)+(dy=+1, 0) into wr; center tap alone in wce
    def pack(w):
        wt = np.transpose(w, (1, 0, 2, 3))  # [ci, co, dy, dx]
        wc = np.empty((128, 3, 64), np.float32)
        for k in range(3):
            wc[0:64, k] = wt[:, :, k, 0]
            wc[64:128, k] = wt[:, :, k, 2]
        wr = np.concatenate([wt[:, :, 0, 1], wt[:, :, 2, 1]], axis=0)
        wce = wt[:, :, 1, 1]
        return wc.astype(BF16), wr.astype(BF16), wce.astype(BF16)

    # baseline 6-pass layout: K-pack dy=-1 (lower) with dy=+1 (upper) per dx
    def pack6(w):
        wt = np.transpose(w, (1, 0, 2, 3))  # [ci, co, dy, dx]
        wp = np.empty((128, 3, 64), np.float32)
        ws = np.empty((64, 3, 64), np.float32)
        for dx in range(3):
            wp[0:64, dx] = wt[:, :, 0, dx]
            wp[64:128, dx] = wt[:, :, 2, dx]
            ws[:, dx] = wt[:, :, 1, dx]
        return wp.astype(BF16), ws.astype(BF16)

    wc1, wr1, wce1 = pack(w1)
    wc2, wr2, wce2 = pack(w2)
    wp1, ws1 = pack6(w1)
    wp2, ws2 = pack6(w2)
    sb1 = np.stack([np.tile(scale1, 2), np.tile(bias1, 2)], axis=1).astype(np.float32)
    sb2 = np.stack([np.tile(scale2, 2), np.tile(bias2, 2)], axis=1).astype(np.float32)

    def flat_padded(m):
        mp = np.zeros((H + 2 * PAD, W), np.float32)
        mp[PAD:PAD + H] = m
        return mp.reshape(-1).astype(BF16)

    gt_list = [flat_padded(gate[b, 0]) for b in range(B)]
    return (wc1, wr1, wce1, wc2, wr2, wce2, wp1, ws1, wp2, ws2,
            sb1, sb2, gt_list)


def kernel(x, gate, w1, scale1, bias1, w2, scale2, bias2):
    from concourse.bass_utils import run_bass_kernel_spmd

    x = np.asarray(x, np.float32)
    gate = np.asarray(gate, np.float32)
    (wc1, wr1, wce1, wc2, wr2, wce2, wp1, ws1, wp2, ws2,
     sb1, sb2, gt_list) = _host_prep(
        gate, np.asarray(w1, np.float32), np.asarray(scale1, np.float32),
        np.asarray(bias1, np.float32), np.asarray(w2, np.float32),
        np.asarray(scale2, np.float32), np.asarray(bias2, np.float32))

    if 'nc' not in _CACHE:
        _CACHE['nc'] = _build()
    nc = _CACHE['nc']

    in_maps = []
    for b in range(B):
        in_maps.append({
            "x": np.ascontiguousarray(x[b]),
            "gt": gt_list[b],
            "wc1": wc1, "wr1": wr1, "wce1": wce1,
            "wc2": wc2, "wr2": wr2, "wce2": wce2,
            "wp1": wp1, "ws1": ws1, "wp2": wp2, "ws2": ws2,
            "sb1": sb1, "sb2": sb2,
        })
    res = run_bass_kernel_spmd(nc, in_maps, core_ids=list(range(B)))
    _CACHE['last_results'] = res
    out = np.stack([np.asarray(res.results[b]["o"], np.float32) for b in range(B)], axis=0)
    return out


# revision 56
# speedup vs baseline: 1.0034x; 1.0013x over previous
"""Trainium2 Bass kernel for masked BasicBlock (conv3x3+BN+ReLU, gated, x2, residual).

Data-parallel over batch: 8 images -> 8 NeuronCores. Per core, NCHW [64,256,256]
in 8 row-strips of 32 output rows:
  - The gate_max mask of the reference is mathematically redundant
    (g*conv2(h*maxpool3(g)) == g*conv2(h)); only the final gate g is applied.
    Out-of-image h rows (h[-1], h[256..]) are zeroed explicitly so conv2 sees
    zero padding.
  - conv3x3 = 5 accumulated matmuls per 2-row group over C_in=64:
      3x K=128 "column pairs" (dy, dx=-1)+(dy, dx=+1) via a 2-col-shifted
        duplicate of the input in partitions 64..127 (tile U / Hu),
      1x K=128 "row pair" (dy=-1, dx=0)+(dy=+1, dx=0) via a 2-row-shifted
        duplicate (tile T1 / H1),
      1x K=64 center tap.
    Chunk pairs (lo|hi = 4 consecutive rows) use the two PE column groups via
    tile_position (0,0)/(0,64).
  - The 2-row shift also makes T1[0:128] directly usable as the residual pair.
  - Gate mask broadcast to all partitions with a stride-0 source AP DMA
    (no PE involvement).
  - BN(+ReLU) on ScalarE from PSUM; elementwise gating and residual on VectorE.
  - Software pipelined: the x staging chain for strip s+1 is prefetched on the
    Pool/SWDGE queues while conv1 of strip s runs, and conv1 of strip s is
    issued before conv2 of strip s-1 so the PE never waits on the h-distribute
    DMAs. First/last strips use a T1/H1-only 6-pass conv form plus per-pair
    distribute/stores to shorten pipeline fill and drain.
"""
import sys
import os

sys.path.insert(0, '/opt/trn_rl_repo')

import numpy as np
import ml_dtypes

BF16 = ml_dtypes.bfloat16

B, C, H, W = 8, 64, 256, 256
WP = W + 2           # padded row width
R = 32               # output rows per strip
NS = H // R          # strips
NP1 = (R + 4) // 4   # conv1 pairs per strip (h rows r0-1 .. r0+34)
NP2 = R // 4         # conv2 pairs per strip
XR = R + 6           # x rows per strip: [r0-2, r0+36)
HR = R + 4           # h rows per strip: [r0-1, r0+35)
PAD = 4              # zero rows padded above/below the flat mask
GTR = R + 1          # gate flat rows loaded per strip

_CACHE = {}


def _build(iters=1):
    import concourse.bacc as bacc_mod
    import concourse.tile as tile
    import concourse.mybir as mybir
    from concourse.bass import AP

    dt = mybir.dt
    nc = bacc_mod.Bacc()

    x_d = nc.dram_tensor("x", [C, H, W], dt.float32, kind="ExternalInput")
    gt_d = nc.dram_tensor("gt", [(H + 2 * PAD) * W], dt.bfloat16, kind="ExternalInput")
    wc1_d = nc.dram_tensor("wc1", [128, 3, 64], dt.bfloat16, kind="ExternalInput")
    wr1_d = nc.dram_tensor("wr1", [128, 64], dt.bfloat16, kind="ExternalInput")
    wce1_d = nc.dram_tensor("wce1", [64, 64], dt.bfloat16, kind="ExternalInput")
    wc2_d = nc.dram_tensor("wc2", [128, 3, 64], dt.bfloat16, kind="ExternalInput")
    wr2_d = nc.dram_tensor("wr2", [128, 64], dt.bfloat16, kind="ExternalInput")
    wce2_d = nc.dram_tensor("wce2", [64, 64], dt.bfloat16, kind="ExternalInput")
    wp1_d = nc.dram_tensor("wp1", [128, 3, 64], dt.bfloat16, kind="ExternalInput")
    ws1_d = nc.dram_tensor("ws1", [64, 3, 64], dt.bfloat16, kind="ExternalInput")
    wp2_d = nc.dram_tensor("wp2", [128, 3, 64], dt.bfloat16, kind="ExternalInput")
    ws2_d = nc.dram_tensor("ws2", [64, 3, 64], dt.bfloat16, kind="ExternalInput")
    sb1_d = nc.dram_tensor("sb1", [128, 2], dt.float32, kind="ExternalInput")
    sb2_d = nc.dram_tensor("sb2", [128, 2], dt.float32, kind="ExternalInput")
    o_d = nc.dram_tensor("o", [C, H, W], dt.bfloat16, kind="ExternalOutput")

    with tile.TileContext(nc) as tc:
        with (
            tc.tile_pool(name="const", bufs=1) as cpool,
            tc.tile_pool(name="xs", bufs=3) as xpool,
            tc.tile_pool(name="us", bufs=2) as upool,
            tc.tile_pool(name="hs", bufs=2) as hpool,
            tc.tile_pool(name="hu", bufs=2) as hupool,
            tc.tile_pool(name="stage", bufs=1) as spool,
            tc.tile_pool(name="ov", bufs=1) as ovpool,
            tc.tile_pool(name="flat", bufs=1) as fpool,
            tc.tile_pool(name="pair", bufs=3) as ppool,
            tc.tile_pool(name="ps1", bufs=4, space="PSUM") as ps1,
            tc.tile_pool(name="ps2", bufs=4, space="PSUM") as ps2,
        ):
            wc1 = cpool.tile([128, 3, 64], dt.bfloat16)
            wr1 = cpool.tile([128, 64], dt.bfloat16)
            wce1 = cpool.tile([64, 64], dt.bfloat16)
            wc2 = cpool.tile([128, 3, 64], dt.bfloat16)
            wr2 = cpool.tile([128, 64], dt.bfloat16)
            wce2 = cpool.tile([64, 64], dt.bfloat16)
            wp1 = cpool.tile([128, 3, 64], dt.bfloat16)
            ws1 = cpool.tile([64, 3, 64], dt.bfloat16)
            wp2 = cpool.tile([128, 3, 64], dt.bfloat16)
            ws2 = cpool.tile([64, 3, 64], dt.bfloat16)
            sb1 = cpool.tile([128, 2], dt.float32)
            sb2 = cpool.tile([128, 2], dt.float32)
            nc.sync.dma_start(wp1[:], wp1_d[:])
            nc.sync.dma_start(ws1[:], ws1_d[:])
            nc.sync.dma_start(wp2[:], wp2_d[:])
            nc.sync.dma_start(ws2[:], ws2_d[:])
            nc.sync.dma_start(wc1[:], wc1_d[:])
            nc.sync.dma_start(wr1[:], wr1_d[:])
            nc.sync.dma_start(wce1[:], wce1_d[:])
            nc.sync.dma_start(wc2[:], wc2_d[:])
            nc.sync.dma_start(wr2[:], wr2_d[:])
            nc.sync.dma_start(wce2[:], wce2_d[:])
            nc.sync.dma_start(sb1[:], sb1_d[:])
            nc.sync.dma_start(sb2[:], sb2_d[:])

            NT = iters * NS

            # x strip staging, all on the Pool queue so the FIFO order matches
            # the dependency chain: load -> row-shift -> col-shift copies.
            # T1 [128, XR, WP] bf16: lower=x padded, upper=x shifted +2 rows.
            # U  [128, XR, 256] bf16: lower=x, upper=x shifted +2 cols.
            def emit_xchain(s):
                r0 = s * R
                T1 = xpool.tile([128, XR, WP], dt.bfloat16, tag="T1")
                first = r0 - 2
                v0 = max(0, -first)
                v1 = min(XR, H - first)
                nc.vector.memset(T1[0:64, :, 0:1], 0)
                nc.vector.memset(T1[0:64, :, 257:258], 0)
                if v0 > 0:
                    nc.vector.memset(T1[0:64, 0:v0, :], 0)
                if v1 < XR:
                    nc.vector.memset(T1[0:64, v1:XR, :], 0)
                if s == 0:
                    # split the first load and shift so the first pairs'
                    # matmuls can start before the whole strip has landed
                    prev_r = v0
                    for mid in (5, 14, 25, XR):
                        nc.gpsimd.dma_start(T1[0:64, prev_r:mid, 1:257], x_d[:, first + prev_r:first + mid, :])
                        nc.gpsimd.dma_start(T1[64:128, max(0, prev_r - 2):mid - 2, :],
                                            T1[0:64, max(2, prev_r):mid, :])
                        prev_r = mid
                    return T1, None  # strip 0 conv1 is 6-pass (T1-only)
                nc.gpsimd.dma_start(T1[0:64, v0:v1, 1:257], x_d[:, first + v0:first + v1, :])
                nc.gpsimd.dma_start(T1[64:128, 0:XR - 2, :], T1[0:64, 2:XR, :])
                if s == 1:
                    return T1, None  # strip 1 conv1 is 6-pass too (lighter fill)
                U = upool.tile([128, XR, 256], dt.bfloat16, tag="U")
                nc.gpsimd.dma_start(U[0:64, :, :], T1[0:64, :, 0:256])
                nc.gpsimd.dma_start(U[64:128, :, :], T1[0:64, :, 2:258])
                return T1, U

            # gate tile [128, NP2, 512]: partition-broadcast via stride-0 src AP;
            # lower partitions get each pair's first 2 rows, upper the next 2
            def emit_gate(s):
                gts = fpool.tile([128, NP2, 512], dt.bfloat16, tag="gts")
                gt0 = (s * R + PAD) * W
                nc.sync.dma_start(gts[0:64, :, :],
                                  AP(gt_d, gt0, [[0, 64], [1024, NP2], [1, 512]]))
                nc.sync.dma_start(gts[64:128, :, :],
                                  AP(gt_d, gt0 + 512, [[0, 64], [1024, NP2], [1, 512]]))
                return gts

            gts_cur = emit_gate(0)

            pending = emit_xchain(0)
            prev = None  # (T1, H1, Hu, s, r0) of strip awaiting conv2
            for it in range(NT + 1):
                if it < NT:
                    s = it % NS
                    r0 = s * R
                    T1, U = pending
                    if it + 1 < NT:
                        pending = emit_xchain((it + 1) % NS)

                    # ---- conv1: 9 pairs of 4 h-rows, 5 passes each half
                    HP = spool.tile([128, NP1, 512], dt.bfloat16, tag="HP")
                    H1 = hpool.tile([128, HR, WP], dt.bfloat16, tag="H1")
                    nc.vector.memset(H1[:, :, 0:1], 0)
                    nc.vector.memset(H1[:, :, 257:258], 0)
                    pp_dist = s in (0, NS - 1)
                    if s > 0:
                        # first 4 h rows (r0-1..r0+3) were already computed by
                        # the previous strip; copy instead of recomputing
                        H1prev = prev[1]
                        nc.sync.dma_start(H1[0:64, 0:4, :], H1prev[0:64, 32:36, :])
                        nc.sync.dma_start(H1[64:128, 0:2, :], H1prev[64:128, 32:34, :])
                    for pq in range(1 if s > 0 else 0, NP1):
                        acc = ps1.tile([128, 512], dt.float32, tag="ps1")
                        b = 4 * pq
                        if s == 0:
                            # T1-only 6-pass form, lower-partition rows first:
                            # no wait on the U col-shift chain or the row-shift
                            # right at kernel start
                            for dx in range(3):
                                nc.tensor.matmul(acc[0:64, :], ws1[:, dx, :], T1[0:64, b + 1:b + 3, dx:dx + 256],
                                                 start=(dx == 0), stop=False, tile_position=(0, 0), skip_group_check=True)
                                nc.tensor.matmul(acc[64:128, :], ws1[:, dx, :], T1[0:64, b + 3:b + 5, dx:dx + 256],
                                                 start=(dx == 0), stop=False, tile_position=(0, 64), skip_group_check=True)
                            for dx in range(3):
                                nc.tensor.matmul(acc[0:64, :], wp1[:, dx, :], T1[:, b:b + 2, dx:dx + 256],
                                                 start=False, stop=(dx == 2), tile_position=(0, 0), skip_group_check=True)
                                nc.tensor.matmul(acc[64:128, :], wp1[:, dx, :], T1[:, b + 2:b + 4, dx:dx + 256],
                                                 start=False, stop=(dx == 2), tile_position=(0, 64), skip_group_check=True)
                            nc.scalar.activation(HP[:, pq, :], acc[:], mybir.ActivationFunctionType.Relu,
                                                 bias=sb1[:, 1:2], scale=sb1[:, 0:1])
                            if pq == 0:
                                nc.vector.memset(HP[0:64, 0, 0:256], 0)   # h row -1
                            hp2 = HP[:, pq, :].rearrange("c (rr w) -> c rr w", rr=2)
                            nc.sync.dma_start(H1[0:64, 4 * pq:4 * pq + 2, 1:257], hp2[0:64])
                            nc.sync.dma_start(H1[0:64, 4 * pq + 2:4 * pq + 4, 1:257], hp2[64:128])
                            if pq > 0:
                                nc.sync.dma_start(H1[64:128, 4 * pq - 2:4 * pq, 1:257], hp2[0:64])
                            nc.sync.dma_start(H1[64:128, 4 * pq:4 * pq + 2, 1:257], hp2[64:128])
                            continue
                        if s == 1:
                            for dx in range(3):
                                nc.tensor.matmul(acc[0:64, :], ws1[:, dx, :], T1[0:64, b + 1:b + 3, dx:dx + 256],
                                                 start=(dx == 0), stop=False, tile_position=(0, 0), skip_group_check=True)
                                nc.tensor.matmul(acc[64:128, :], ws1[:, dx, :], T1[0:64, b + 3:b + 5, dx:dx + 256],
                                                 start=(dx == 0), stop=False, tile_position=(0, 64), skip_group_check=True)
                            for dx in range(3):
                                nc.tensor.matmul(acc[0:64, :], wp1[:, dx, :], T1[:, b:b + 2, dx:dx + 256],
                                                 start=False, stop=(dx == 2), tile_position=(0, 0), skip_group_check=True)
                                nc.tensor.matmul(acc[64:128, :], wp1[:, dx, :], T1[:, b + 2:b + 4, dx:dx + 256],
                                                 start=False, stop=(dx == 2), tile_position=(0, 64), skip_group_check=True)
                        else:
                            nc.tensor.matmul(acc[0:64, :], wce1[:], T1[0:64, b + 1:b + 3, 1:257],
                                             start=True, stop=False, tile_position=(0, 0), skip_group_check=True)
                            nc.tensor.matmul(acc[64:128, :], wce1[:], T1[0:64, b + 3:b + 5, 1:257],
                                             start=True, stop=False, tile_position=(0, 64), skip_group_check=True)
                            nc.tensor.matmul(acc[0:64, :], wr1[:], T1[:, b:b + 2, 1:257],
                                             start=False, stop=False, tile_position=(0, 0), skip_group_check=True)
                            nc.tensor.matmul(acc[64:128, :], wr1[:], T1[:, b + 2:b + 4, 1:257],
                                             start=False, stop=False, tile_position=(0, 64), skip_group_check=True)
                            for k in range(3):
                                nc.tensor.matmul(acc[0:64, :], wc1[:, k, :], U[:, b + k:b + k + 2, :],
                                                 start=False, stop=(k == 2), tile_position=(0, 0), skip_group_check=True)
                                nc.tensor.matmul(acc[64:128, :], wc1[:, k, :], U[:, b + 2 + k:b + 4 + k, :],
                                                 start=False, stop=(k == 2), tile_position=(0, 64), skip_group_check=True)
                        nc.scalar.activation(HP[:, pq, :], acc[:], mybir.ActivationFunctionType.Relu,
                                             bias=sb1[:, 1:2], scale=sb1[:, 0:1])
                        # zero h rows outside the image so conv2 sees zero padding
                        if s == NS - 1 and pq == NP1 - 1:
                            nc.vector.memset(HP[0:64, NP1 - 1, 256:512], 0)  # h row 256
                            nc.vector.memset(HP[64:128, NP1 - 1, :], 0)      # h rows 257,258
                        if pp_dist:
                            hp2 = HP[:, pq, :].rearrange("c (rr w) -> c rr w", rr=2)
                            nc.sync.dma_start(H1[0:64, 4 * pq:4 * pq + 2, 1:257], hp2[0:64])
                            nc.sync.dma_start(H1[0:64, 4 * pq + 2:4 * pq + 4, 1:257], hp2[64:128])
                            if pq > 0:
                                nc.sync.dma_start(H1[64:128, 4 * pq - 2:4 * pq, 1:257], hp2[0:64])
                            nc.sync.dma_start(H1[64:128, 4 * pq:4 * pq + 2, 1:257], hp2[64:128])

                    if not pp_dist:
                        # distribute HP -> H1 for pairs 1..NP1-1 (lower = h,
                        # upper = h shifted +2 rows); DMA APs max 3 dims: one
                        # DMA per row-in-pair
                        np1 = NP1 - 1
                        h1v = H1[0:64, 4:4 + 4 * np1, 1:257].rearrange("c (p rr) w -> c p rr w", p=np1)
                        h1u = H1[64:128, 2:2 + 4 * np1, 1:257].rearrange("c (p rr) w -> c p rr w", p=np1)
                        h1u2 = H1[64:128, 4:4 + 4 * np1, 1:257].rearrange("c (p rr) w -> c p rr w", p=np1)
                        hpv = HP[:].rearrange("c p (rr w) -> c p rr w", rr=2)
                        for rr in range(2):
                            nc.sync.dma_start(h1v[:, :, rr, :], hpv[0:64, 1:NP1, rr, :])
                            nc.sync.dma_start(h1v[:, :, 2 + rr, :], hpv[64:128, 1:NP1, rr, :])
                            nc.sync.dma_start(h1u[:, :, rr, :], hpv[0:64, 1:NP1, rr, :])
                            nc.sync.dma_start(h1u2[:, :, rr, :], hpv[64:128, 1:NP1, rr, :])
                    if s != NS - 1:
                        # Hu: lower = h, upper = h shifted +2 cols (5-pass conv2)
                        Hu = hupool.tile([128, HR, 256], dt.bfloat16, tag="Hu")
                        nc.sync.dma_start(Hu[0:64, :, :], H1[0:64, :, 0:256])
                        nc.sync.dma_start(Hu[64:128, :, :], H1[0:64, :, 2:258])
                    else:
                        Hu = None

                    cur = (T1, H1, Hu, s, r0)
                else:
                    cur = None

                if prev is not None:
                    T1p, H1p, Hup, sp, r0p = prev
                    # ---- conv2: 8 pairs of 4 output rows, 5 passes each half
                    OV = ovpool.tile([128, NP2, 512], dt.bfloat16, tag="OV")
                    for q in range(NP2):
                        b = 4 * q
                        acc2 = ps2.tile([128, 512], dt.float32, tag="ps2")
                        if sp == NS - 1:
                            # H1-only 6-pass form: no wait on the Hu col-shift
                            # chain in the pipeline drain
                            for dx in range(3):
                                nc.tensor.matmul(acc2[0:64, :], wp2[:, dx, :], H1p[:, b:b + 2, dx:dx + 256],
                                                 start=(dx == 0), stop=False, tile_position=(0, 0), skip_group_check=True)
                                nc.tensor.matmul(acc2[64:128, :], wp2[:, dx, :], H1p[:, b + 2:b + 4, dx:dx + 256],
                                                 start=(dx == 0), stop=False, tile_position=(0, 64), skip_group_check=True)
                            for dx in range(3):
                                nc.tensor.matmul(acc2[0:64, :], ws2[:, dx, :], H1p[0:64, b + 1:b + 3, dx:dx + 256],
                                                 start=False, stop=(dx == 2), tile_position=(0, 0), skip_group_check=True)
                                nc.tensor.matmul(acc2[64:128, :], ws2[:, dx, :], H1p[0:64, b + 3:b + 5, dx:dx + 256],
                                                 start=False, stop=(dx == 2), tile_position=(0, 64), skip_group_check=True)
                        else:
                            nc.tensor.matmul(acc2[0:64, :], wce2[:], H1p[0:64, b + 1:b + 3, 1:257],
                                             start=True, stop=False, tile_position=(0, 0), skip_group_check=True)
                            nc.tensor.matmul(acc2[64:128, :], wce2[:], H1p[0:64, b + 3:b + 5, 1:257],
                                             start=True, stop=False, tile_position=(0, 64), skip_group_check=True)
                            nc.tensor.matmul(acc2[0:64, :], wr2[:], H1p[:, b:b + 2, 1:257],
                                             start=False, stop=False, tile_position=(0, 0), skip_group_check=True)
                            nc.tensor.matmul(acc2[64:128, :], wr2[:], H1p[:, b + 2:b + 4, 1:257],
                                             start=False, stop=False, tile_position=(0, 64), skip_group_check=True)
                            for k in range(3):
                                nc.tensor.matmul(acc2[0:64, :], wc2[:, k, :], Hup[:, b + k:b + k + 2, :],
                                                 start=False, stop=(k == 2), tile_position=(0, 0), skip_group_check=True)
                                nc.tensor.matmul(acc2[64:128, :], wc2[:, k, :], Hup[:, b + 2 + k:b + 4 + k, :],
                                                 start=False, stop=(k == 2), tile_position=(0, 64), skip_group_check=True)

                        u2 = ppool.tile([128, 512], dt.bfloat16, tag="u2")
                        nc.scalar.activation(u2[:], acc2[:], mybir.ActivationFunctionType.Identity,
                                             bias=sb2[:, 1:2], scale=sb2[:, 0:1])
                        nc.vector.tensor_tensor(u2[:], u2[:], gts_cur[:, q, :], mybir.AluOpType.mult)
                        lz = b + 2
                        nc.vector.tensor_tensor(u2[:].rearrange("p (r w) -> p r w", r=2),
                                                u2[:].rearrange("p (r w) -> p r w", r=2),
                                                T1p[:, lz:lz + 2, 1:257], mybir.AluOpType.add)
                        nc.scalar.activation(OV[:, q, :], u2[:], mybir.ActivationFunctionType.Relu)
                        if sp == NS - 1:
                            ovq = OV[:, q, :].rearrange("c (rr w) -> c rr w", rr=2)
                            nc.sync.dma_start(o_d[:, r0p + b:r0p + b + 2, :], ovq[0:64])
                            nc.sync.dma_start(o_d[:, r0p + b + 2:r0p + b + 4, :], ovq[64:128])

                    if sp != NS - 1:
                        ov = o_d[:, r0p:r0p + R, :].rearrange("c (p rr) w -> c p rr w", p=NP2)
                        ovv = OV[:].rearrange("c p (rr w) -> c p rr w", rr=2)
                        for rr in range(2):
                            nc.scalar.dma_start(ov[:, :, rr, :], ovv[0:64, :, rr, :])
                            nc.scalar.dma_start(ov[:, :, 2 + rr, :], ovv[64:128, :, rr, :])

                if 0 < it < NT:
                    # gate for iteration it+1's conv2 (bufs=1: WAR on this
                    # iteration's conv2 mask reads orders this load after them)
                    gts_cur = emit_gate(it % NS)

                prev = cur
    nc.finalize()
    return nc


def _host_prep(gate, w1, scale1, bias1, w2, scale2, bias2):
    # lhsT[ci, co] = w[co, ci, dy, dx]; K-pack (dy, dx=-1)+(dy, dx=+1) into wc,
    # (dy=-1, 0)+(dy=+1, 0) into wr; center tap alone in wce
    def pack(w):
        wt = np.transpose(w, (1, 0, 2, 3))  # [ci, co, dy, dx]
        wc = np.empty((128, 3, 64), np.float32)
        for k in range(3):
            wc[0:64, k] = wt[:, :, k, 0]
            wc[64:128, k] = wt[:, :, k, 2]
        wr = np.concatenate([wt[:, :, 0, 1], wt[:, :, 2, 1]], axis=0)
        wce = wt[:, :, 1, 1]
        return wc.astype(BF16), wr.astype(BF16), wce.astype(BF16)

    # baseline 6-pass layout: K-pack dy=-1 (lower) with dy=+1 (upper) per dx
    def pack6(w):
        wt = np.transpose(w, (1, 0, 2, 3))  # [ci, co, dy, dx]
        wp = np.empty((128, 3, 64), np.float32)
        ws = np.empty((64, 3, 64), np.float32)
        for dx in range(3):
            wp[0:64, dx] = wt[:, :, 0, dx]
            wp[64:128, dx] = wt[:, :, 2, dx]
            ws[:, dx] = wt[:, :, 1, dx]
        return wp.astype(BF16), ws.astype(BF16)

    wc1, wr1, wce1 = pack(w1)
    wc2, wr2, wce2 = pack(w2)
    wp1, ws1 = pack6(w1)
    wp2, ws2 = pack6(w2)
    sb1 = np.stack([np.tile(scale1, 2), np.tile(bias1, 2)], axis=1).astype(np.float32)
    sb2 = np.stack([np.tile(scale2, 2), np.tile(bias2, 2)], axis=1).astype(np.float32)

    def flat_padded(m):
        mp = np.zeros((H + 2 * PAD, W), np.float32)
        mp[PAD:PAD + H] = m
        return mp.reshape(-1).astype(BF16)

    gt_list = [flat_padded(gate[b, 0]) for b in range(B)]
    return (wc1, wr1, wce1, wc2, wr2, wce2, wp1, ws1, wp2, ws2,
            sb1, sb2, gt_list)


def kernel(x, gate, w1, scale1, bias1, w2, scale2, bias2):
    from concourse.bass_utils import run_bass_kernel_spmd

    x = np.asarray(x, np.float32)
    gate = np.asarray(gate, np.float32)
    (wc1, wr1, wce1, wc2, wr2, wce2, wp1, ws1, wp2, ws2,
     sb1, sb2, gt_list) = _host_prep(
        gate, np.asarray(w1, np.float32), np.asarray(scale1, np.float32),
        np.asarray(bias1, np.float32), np.asarray(w2, np.float32),
        np.asarray(scale2, np.float32), np.asarray(bias2, np.float32))

    if 'nc' not in _CACHE:
        _CACHE['nc'] = _build()
    nc = _CACHE['nc']

    in_maps = []
    for b in range(B):
        in_maps.append({
            "x": np.ascontiguousarray(x[b]),
            "gt": gt_list[b],
            "wc1": wc1, "wr1": wr1, "wce1": wce1,
            "wc2": wc2, "wr2": wr2, "wce2": wce2,
            "wp1": wp1, "ws1": ws1, "wp2": wp2, "ws2": ws2,
            "sb1": sb1, "sb2": sb2,
        })
    res = run_bass_kernel_spmd(nc, in_maps, core_ids=list(range(B)))
    _CACHE['last_results'] = res
    out = np.stack([np.asarray(res.results[b]["o"], np.float32) for b in range(B)], axis=0)
    return out
